# revision 8
# baseline (speedup 1.0000x reference)
"""Multi-head attention with relative position embeddings, on 8 NeuronCores.

Sharding: tensor-parallel over heads. Core h handles head h: it gets the
full query/key/value plus the per-head slices of Wq/Wk/Wv/Wo and
relative_k/relative_v. Each core computes attn_h [512,512] and the partial
output projection context_h @ Wo_h [512,512] (transposed); the host sums the
8 partials (+ bo) and stacks the attn maps.

Per-core device schedule:
  - PE transposes query/key/value, does head projections, q@k^T, attn@v and
    the Wo partial.
  - The relative einsums stream rel_k/rel_v from HBM in 2MB tiles
    [128 l, 64 m, 64 d]; DVE does a broadcast multiply + an innermost-axis
    reduce per tile.
  - ACT does softmax exp and PSUM->SBUF copies.
"""

import sys

sys.path.insert(0, "/opt/trn_rl_repo")

import numpy as np

L = 512  # sequence length
DM = 512  # model dim
H = 8  # heads
D = 64  # dim per head
NC_ = 4  # number of 128-row chunks of L
G = 64  # m-group size for rel tensor streaming
NG = L // G  # 8 groups per chunk

_cache = {}


def _build():
    from contextlib import ExitStack

    import concourse.bacc as bacc
    import concourse.bass as bass
    import concourse.tile as tile
    from concourse import mybir
    from concourse.masks import make_identity

    f32 = mybir.dt.float32
    AX = mybir.AxisListType
    OP = mybir.AluOpType
    AF = mybir.ActivationFunctionType

    nc = bacc.Bacc("TRN2", target_bir_lowering=False)

    xq = nc.dram_tensor("xq", [L, DM], f32, kind="ExternalInput")
    xk = nc.dram_tensor("xk", [L, DM], f32, kind="ExternalInput")
    xv = nc.dram_tensor("xv", [L, DM], f32, kind="ExternalInput")
    wq = nc.dram_tensor("wq", [DM, D], f32, kind="ExternalInput")
    wk = nc.dram_tensor("wk", [DM, D], f32, kind="ExternalInput")
    wv = nc.dram_tensor("wv", [DM, D], f32, kind="ExternalInput")
    bqs = nc.dram_tensor("bqs", [D, 1], f32, kind="ExternalInput")
    bks = nc.dram_tensor("bks", [D, 1], f32, kind="ExternalInput")
    bvs = nc.dram_tensor("bvs", [D, 1], f32, kind="ExternalInput")
    wo = nc.dram_tensor("wo", [D, DM], f32, kind="ExternalInput")
    rk = nc.dram_tensor("rk", [L, L, D], f32, kind="ExternalInput")
    rv = nc.dram_tensor("rv", [L, L, D], f32, kind="ExternalInput")

    attn_o = nc.dram_tensor("attn_o", [L, L], f32, kind="ExternalOutput")
    pout_t = nc.dram_tensor("pout_t", [DM, L], f32, kind="ExternalOutput")

    with ExitStack() as ctx:
        tc = ctx.enter_context(tile.TileContext(nc))
        const = ctx.enter_context(tc.tile_pool(name="const", bufs=1))
        persist = ctx.enter_context(tc.tile_pool(name="persist", bufs=1))
        psA = ctx.enter_context(tc.tile_pool(name="psA", bufs=2, space="PSUM"))
        psB = ctx.enter_context(tc.tile_pool(name="psB", bufs=2, space="PSUM"))
        psC = ctx.enter_context(tc.tile_pool(name="psC", bufs=1, space="PSUM"))
        psD = ctx.enter_context(tc.tile_pool(name="psD", bufs=2, space="PSUM"))

        ident = const.tile([128, 128], f32)
        make_identity(nc, ident[:, :])

        wq_sb = const.tile([128, 4, D], f32)
        wk_sb = const.tile([128, 4, D], f32)
        wv_sb = const.tile([128, 4, D], f32)
        nc.sync.dma_start(wq_sb[:, :, :], wq[:, :].rearrange("(c p) d -> p c d", c=4))
        nc.sync.dma_start(wk_sb[:, :, :], wk[:, :].rearrange("(c p) d -> p c d", c=4))
        nc.sync.dma_start(wv_sb[:, :, :], wv[:, :].rearrange("(c p) d -> p c d", c=4))
        wo_sb = const.tile([D, DM], f32)
        nc.sync.dma_start(wo_sb[:, :], wo[:, :])
        bq_sb = const.tile([D, 1], f32)
        bk_sb = const.tile([D, 1], f32)
        bv_sb = const.tile([D, 1], f32)
        nc.sync.dma_start(bq_sb[:, :], bqs[:, :])
        nc.sync.dma_start(bk_sb[:, :], bks[:, :])
        nc.sync.dma_start(bv_sb[:, :], bvs[:, :])

        # Persistent head tensors.
        q_hT = persist.tile([D, L], f32)  # q^T/sqrt(D): [dh, l]
        k_hT = persist.tile([D, L], f32)  # k^T: [dh, m]
        q_sb = persist.tile([128, 4, D], f32)  # q natural: [l-part, lc, dh]
        v_sb = persist.tile([128, 4, D], f32)  # v natural: [m-part, mc, dh]
        attn_sb = persist.tile([128, 4, L], f32)  # attn: [l-part, lc, m]
        attnT = persist.tile([128, 4, L], f32)  # attn^T: [m-part, mc, l]
        ctxT = persist.tile([D, L], f32)  # context^T: [dh, l]

        # ---- Stage 0: load q/k/v, transpose, project ----
        with tc.tile_pool(name="s0", bufs=1) as s0:
            x_sb = {}
            xT = {}
            for name, t in (("q", xq), ("k", xk), ("v", xv)):
                xs = s0.tile([128, 4, DM], f32, tag=f"x_{name}", name=f"x_{name}")
                nc.sync.dma_start(
                    xs[:, :, :], t[:, :].rearrange("(c p) d -> p c d", c=4)
                )
                x_sb[name] = xs
                xT[name] = s0.tile(
                    [128, 4, L], f32, tag=f"xT_{name}", name=f"xT_{name}"
                )
            for name in ("q", "k", "v"):
                for c in range(4):  # l-chunk
                    for j in range(4):  # dm-chunk
                        pt = psB.tile([128, 128], f32, tag="ptr")
                        nc.tensor.transpose(
                            pt[:, :],
                            x_sb[name][:, c, j * 128 : (j + 1) * 128],
                            ident[:, :],
                        )
                        nc.scalar.copy(
                            xT[name][:, j, c * 128 : (c + 1) * 128], pt[:, :]
                        )

            # Projections into transposed head layout [dh, l].
            v_hT = s0.tile([D, L], f32)
            for dst, w_sb, xn, b_sb, scale in (
                (q_hT, wq_sb, "q", bq_sb, 0.125),
                (k_hT, wk_sb, "k", bk_sb, 1.0),
                (v_hT, wv_sb, "v", bv_sb, 1.0),
            ):
                pq = psC.tile([D, L], f32, tag="proj")
                for c in range(4):
                    nc.tensor.matmul(
                        pq[:, :],
                        wq_sb if False else w_sb[:, c, :],
                        xT[xn][:, c, :],
                        start=(c == 0),
                        stop=(c == 3),
                    )
                nc.scalar.activation(
                    dst[:, :], pq[:, :], AF.Identity, bias=b_sb[:, :], scale=scale
                )

            # q natural [l, dh] and v natural [m, dh] via PE transpose.
            for src, dst in ((q_hT, q_sb), (v_hT, v_sb)):
                for c in range(4):
                    pt = psB.tile([128, D], f32, tag="ptr", name="pt")
                    nc.tensor.transpose(
                        pt[:, :], src[:, c * 128 : (c + 1) * 128], ident[:D, :D]
                    )
                    nc.scalar.copy(dst[:, c, :], pt[:, :])

        # ---- Main loop over l-chunks ----
        rk_pool = ctx.enter_context(tc.tile_pool(name="rkp", bufs=3))
        rv_pool = ctx.enter_context(tc.tile_pool(name="rvp", bufs=3))
        small = ctx.enter_context(tc.tile_pool(name="small", bufs=2))
        outp = ctx.enter_context(tc.tile_pool(name="outp", bufs=2))

        for lc in range(4):
            lsl = slice(lc * 128, (lc + 1) * 128)

            # logits main term: q_h @ k_h^T -> [128 l, 512 m] in PSUM
            pl = psA.tile([128, L], f32, tag="logits")
            nc.tensor.matmul(
                pl[:, :], q_hT[:, lsl], k_hT[:, :], start=True, stop=True
            )

            # rel-k term on DVE
            lrel = small.tile([128, L], f32, tag="lrel")
            for g in range(NG):
                rkt = rk_pool.tile([128, G, D], f32, tag="rkt")
                nc.sync.dma_start(
                    rkt[:, :, :], rk[lsl, g * G : (g + 1) * G, :]
                )
                qb = q_sb[:, lc, :].unsqueeze(1).broadcast_to([128, G, D])
                nc.vector.tensor_tensor(
                    rkt[:, :, :], rkt[:, :, :], qb, OP.mult
                )
                nc.vector.tensor_reduce(
                    lrel[:, g * G : (g + 1) * G], rkt[:, :, :], AX.X, OP.add
                )

            logits = small.tile([128, L], f32, tag="logits_sb")
            nc.vector.tensor_tensor(logits[:, :], pl[:, :], lrel[:, :], OP.add)

            # softmax
            mx = small.tile([128, 1], f32, tag="mx")
            nmx = small.tile([128, 1], f32, tag="nmx")
            sm = small.tile([128, 1], f32, tag="sm")
            rs = small.tile([128, 1], f32, tag="rs")
            nc.vector.tensor_reduce(mx[:, :], logits[:, :], AX.X, OP.max)
            nc.vector.tensor_scalar_mul(nmx[:, :], mx[:, :], -1.0)
            nc.scalar.activation(
                attn_sb[:, lc, :], logits[:, :], AF.Exp, bias=nmx[:, :], scale=1.0
            )
            nc.vector.tensor_reduce(sm[:, :], attn_sb[:, lc, :], AX.X, OP.add)
            nc.vector.reciprocal(rs[:, :], sm[:, :])
            nc.vector.tensor_scalar_mul(attn_sb[:, lc, :], attn_sb[:, lc, :], rs[:, :])

            nc.sync.dma_start(attn_o[lsl, :], attn_sb[:, lc, :])

            # attn^T blocks for the context matmul
            for j in range(4):
                pt = psB.tile([128, 128], f32, tag="ptr")
                nc.tensor.transpose(
                    pt[:, :], attn_sb[:, lc, j * 128 : (j + 1) * 128], ident[:, :]
                )
                nc.scalar.copy(attnT[:, j, lsl], pt[:, :])

            # rel-v term on DVE
            ctx_rel = small.tile([128, D], f32, tag="ctx_rel")
            rtmp = small.tile([128, D], f32, tag="rtmp")
            for g in range(NG):
                rvt = rv_pool.tile([128, G, D], f32, tag="rvt")
                nc.sync.dma_start(
                    rvt[:, :, :], rv[lsl, g * G : (g + 1) * G, :]
                )
                ab = (
                    attn_sb[:, lc, g * G : (g + 1) * G]
                    .unsqueeze(2)
                    .broadcast_to([128, G, D])
                )
                nc.vector.tensor_tensor(rvt[:, :, :], rvt[:, :, :], ab, OP.mult)
                red_dst = ctx_rel if g == 0 else rtmp
                nc.vector.tensor_reduce(
                    red_dst[:, :],
                    rvt[:, :, :].rearrange("p m d -> p d m"),
                    AX.X,
                    OP.add,
                )
                if g > 0:
                    nc.vector.tensor_tensor(
                        ctx_rel[:, :], ctx_rel[:, :], rtmp[:, :], OP.add
                    )

            # context main term: attn @ v -> [128 l, 64]
            pc = psD.tile([128, D], f32, tag="pctx")
            for j in range(4):
                nc.tensor.matmul(
                    pc[:, :],
                    attnT[:, j, lsl],
                    v_sb[:, j, :],
                    start=(j == 0),
                    stop=(j == 3),
                )
            ctx_sb = small.tile([128, D], f32, tag="ctx_sb")
            nc.vector.tensor_tensor(ctx_sb[:, :], pc[:, :], ctx_rel[:, :], OP.add)

            # context^T [dh, l-slice]
            ptc = psB.tile([D, 128], f32, tag="ptr", name="ptc")
            nc.tensor.transpose(ptc[:, :], ctx_sb[:, :], ident[:, :])
            nc.scalar.copy(ctxT[:, lsl], ptc[:, :])

        # ---- Output projection partial: (ctx @ Wo)^T = Wo^T-slices @ ctxT ----
        for c in range(4):
            po = psA.tile([128, L], f32, tag="logits")
            nc.tensor.matmul(
                po[:, :],
                wo_sb[:, c * 128 : (c + 1) * 128],
                ctxT[:, :],
                start=True,
                stop=True,
            )
            osb = outp.tile([128, L], f32, tag="osb")
            nc.scalar.copy(osb[:, :], po[:, :])
            nc.sync.dma_start(pout_t[c * 128 : (c + 1) * 128, :], osb[:, :])

    nc.finalize()
    return nc


def _get_nc():
    if "nc" not in _cache:
        _cache["nc"] = _build()
    return _cache["nc"]


def _prep_in_maps(
    query, key, value, relative_k, relative_v, Wq, bq, Wk, bk, Wv, bv, Wo
):
    q = np.ascontiguousarray(np.asarray(query, np.float32).reshape(L, DM))
    k = np.ascontiguousarray(np.asarray(key, np.float32).reshape(L, DM))
    v = np.ascontiguousarray(np.asarray(value, np.float32).reshape(L, DM))
    rk = np.ascontiguousarray(np.asarray(relative_k, np.float32).reshape(H, L, L, D))
    rv = np.ascontiguousarray(np.asarray(relative_v, np.float32).reshape(H, L, L, D))
    Wq = np.asarray(Wq, np.float32)
    Wk = np.asarray(Wk, np.float32)
    Wv = np.asarray(Wv, np.float32)
    Wo = np.asarray(Wo, np.float32)
    bq = np.asarray(bq, np.float32)
    bk = np.asarray(bk, np.float32)
    bv = np.asarray(bv, np.float32)

    in_maps = []
    for h in range(H):
        sl = slice(h * D, (h + 1) * D)
        in_maps.append(
            {
                "xq": q,
                "xk": k,
                "xv": v,
                "wq": np.ascontiguousarray(Wq[:, sl]),
                "wk": np.ascontiguousarray(Wk[:, sl]),
                "wv": np.ascontiguousarray(Wv[:, sl]),
                "bqs": np.ascontiguousarray((bq[sl] / 8.0).reshape(D, 1)),
                "bks": np.ascontiguousarray(bk[sl].reshape(D, 1)),
                "bvs": np.ascontiguousarray(bv[sl].reshape(D, 1)),
                "wo": np.ascontiguousarray(Wo[sl, :]),
                "rk": rk[h],
                "rv": rv[h],
            }
        )
    return in_maps


def _assemble(res, bo):
    bo = np.asarray(bo, np.float32)
    attn = np.stack([r["attn_o"] for r in res.results])[None]
    out = sum(r["pout_t"].T for r in res.results) + bo
    return out[None].astype(np.float32), attn.astype(np.float32)


def kernel(
    query,
    key,
    value,
    relative_k,
    relative_v,
    Wq,
    bq,
    Wk,
    bk,
    Wv,
    bv,
    Wo,
    bo,
):
    from concourse.bass_utils import run_bass_kernel_spmd

    in_maps = _prep_in_maps(
        query, key, value, relative_k, relative_v, Wq, bq, Wk, bk, Wv, bv, Wo
    )
    res = run_bass_kernel_spmd(_get_nc(), in_maps, core_ids=list(range(H)))
    return _assemble(res, bo)


def profile(inputs, tmpdir=None):
    """Run once with NTFF tracing; returns (outputs, exec_time_ns, results)."""
    from concourse.bass_utils import run_bass_kernel_spmd

    in_maps = _prep_in_maps(
        **{k: v for k, v in inputs.items() if k not in ("bo",)}
    )
    res = run_bass_kernel_spmd(
        _get_nc(), in_maps, core_ids=list(range(H)), trace=True, tmpdir=tmpdir
    )
    outs = _assemble(res, inputs["bo"])
    return outs, res.exec_time_ns, res


# revision 9
# speedup vs baseline: 132.4627x; 132.4627x over previous
"""Multi-head attention with relative position embeddings, on 8 NeuronCores.

Sharding: tensor-parallel over heads. Core h handles head h: it gets the
full query/key/value plus the per-head slices of Wq/Wk/Wv/Wo and
relative_k/relative_v. Each core computes attn_h [512,512] and the partial
output projection context_h @ Wo_h [512,512] (transposed); the host sums the
8 partials (+ bo) and stacks the attn maps.

Per-core device schedule:
  - PE transposes query/key/value, does head projections, q@k^T, attn@v and
    the Wo partial.
  - The relative einsums stream rel_k/rel_v from HBM in 2MB tiles
    [128 l, 64 m, 64 d]; DVE does a broadcast multiply + an innermost-axis
    reduce per tile.
  - ACT does softmax exp and PSUM->SBUF copies.
"""

import sys

sys.path.insert(0, "/opt/trn_rl_repo")

import numpy as np

L = 512  # sequence length
DM = 512  # model dim
H = 8  # heads
D = 64  # dim per head
NC_ = 4  # number of 128-row chunks of L
G = 64  # m-group size for rel tensor streaming
NG = L // G  # 8 groups per chunk

_cache = {}


def _build(repeat=1):
    from contextlib import ExitStack

    import concourse.bacc as bacc
    import concourse.bass as bass
    import concourse.tile as tile
    from concourse import mybir
    from concourse.masks import make_identity

    f32 = mybir.dt.float32
    AX = mybir.AxisListType
    OP = mybir.AluOpType
    AF = mybir.ActivationFunctionType

    nc = bacc.Bacc("TRN2", target_bir_lowering=False)

    xq = nc.dram_tensor("xq", [L, DM], f32, kind="ExternalInput")
    xk = nc.dram_tensor("xk", [L, DM], f32, kind="ExternalInput")
    xv = nc.dram_tensor("xv", [L, DM], f32, kind="ExternalInput")
    wq = nc.dram_tensor("wq", [DM, D], f32, kind="ExternalInput")
    wk = nc.dram_tensor("wk", [DM, D], f32, kind="ExternalInput")
    wv = nc.dram_tensor("wv", [DM, D], f32, kind="ExternalInput")
    bqs = nc.dram_tensor("bqs", [D, 1], f32, kind="ExternalInput")
    bks = nc.dram_tensor("bks", [D, 1], f32, kind="ExternalInput")
    bvs = nc.dram_tensor("bvs", [D, 1], f32, kind="ExternalInput")
    wo = nc.dram_tensor("wo", [D, DM], f32, kind="ExternalInput")
    rk = nc.dram_tensor("rk", [L, L, D], f32, kind="ExternalInput")
    rv = nc.dram_tensor("rv", [L, L, D], f32, kind="ExternalInput")

    attn_o = nc.dram_tensor("attn_o", [L, L], f32, kind="ExternalOutput")
    pout_t = nc.dram_tensor("pout_t", [DM, L], f32, kind="ExternalOutput")

    with ExitStack() as ctx:
        tc = ctx.enter_context(tile.TileContext(nc))
        const = ctx.enter_context(tc.tile_pool(name="const", bufs=1))
        persist = ctx.enter_context(tc.tile_pool(name="persist", bufs=1))
        psA = ctx.enter_context(tc.tile_pool(name="psA", bufs=2, space="PSUM"))
        psB = ctx.enter_context(tc.tile_pool(name="psB", bufs=2, space="PSUM"))
        psC = ctx.enter_context(tc.tile_pool(name="psC", bufs=1, space="PSUM"))
        psD = ctx.enter_context(tc.tile_pool(name="psD", bufs=2, space="PSUM"))

        ident = const.tile([128, 128], f32)
        make_identity(nc, ident[:, :])

        wq_sb = const.tile([128, 4, D], f32)
        wk_sb = const.tile([128, 4, D], f32)
        wv_sb = const.tile([128, 4, D], f32)
        nc.sync.dma_start(wq_sb[:, :, :], wq[:, :].rearrange("(c p) d -> p c d", c=4))
        nc.sync.dma_start(wk_sb[:, :, :], wk[:, :].rearrange("(c p) d -> p c d", c=4))
        nc.sync.dma_start(wv_sb[:, :, :], wv[:, :].rearrange("(c p) d -> p c d", c=4))
        wo_sb = const.tile([D, DM], f32)
        nc.sync.dma_start(wo_sb[:, :], wo[:, :])
        bq_sb = const.tile([D, 1], f32)
        bk_sb = const.tile([D, 1], f32)
        bv_sb = const.tile([D, 1], f32)
        nc.sync.dma_start(bq_sb[:, :], bqs[:, :])
        nc.sync.dma_start(bk_sb[:, :], bks[:, :])
        nc.sync.dma_start(bv_sb[:, :], bvs[:, :])

        # Persistent head tensors.
        q_hT = persist.tile([D, L], f32)  # q^T/sqrt(D): [dh, l]
        k_hT = persist.tile([D, L], f32)  # k^T: [dh, m]
        q_sb = persist.tile([128, 4, D], f32)  # q natural: [l-part, lc, dh]
        v_sb = persist.tile([128, 4, D], f32)  # v natural: [m-part, mc, dh]
        attn_sb = persist.tile([128, 4, L], f32)  # attn: [l-part, lc, m]
        attnT = persist.tile([128, 4, L], f32)  # attn^T: [m-part, mc, l]
        ctxT = persist.tile([D, L], f32)  # context^T: [dh, l]

        # Optional on-device repeat loop for benchmarking (repeat>1).
        rep_cm = tc.For_i(0, repeat, 1) if repeat > 1 else None
        if rep_cm is not None:
            ctx.enter_context(rep_cm)

        # ---- Stage 0: load q/k/v, transpose, project ----
        with tc.tile_pool(name="s0", bufs=1) as s0:
            x_sb = {}
            xT = {}
            for name, t in (("q", xq), ("k", xk), ("v", xv)):
                xs = s0.tile([128, 4, DM], f32, tag=f"x_{name}", name=f"x_{name}")
                nc.sync.dma_start(
                    xs[:, :, :], t[:, :].rearrange("(c p) d -> p c d", c=4)
                )
                x_sb[name] = xs
                xT[name] = s0.tile(
                    [128, 4, L], f32, tag=f"xT_{name}", name=f"xT_{name}"
                )
            for name in ("q", "k", "v"):
                for c in range(4):  # l-chunk
                    for j in range(4):  # dm-chunk
                        pt = psB.tile([128, 128], f32, tag="ptr")
                        nc.tensor.transpose(
                            pt[:, :],
                            x_sb[name][:, c, j * 128 : (j + 1) * 128],
                            ident[:, :],
                        )
                        nc.scalar.copy(
                            xT[name][:, j, c * 128 : (c + 1) * 128], pt[:, :]
                        )

            # Projections into transposed head layout [dh, l].
            v_hT = s0.tile([D, L], f32)
            for dst, w_sb, xn, b_sb, scale in (
                (q_hT, wq_sb, "q", bq_sb, 0.125),
                (k_hT, wk_sb, "k", bk_sb, 1.0),
                (v_hT, wv_sb, "v", bv_sb, 1.0),
            ):
                pq = psC.tile([D, L], f32, tag="proj")
                for c in range(4):
                    nc.tensor.matmul(
                        pq[:, :],
                        wq_sb if False else w_sb[:, c, :],
                        xT[xn][:, c, :],
                        start=(c == 0),
                        stop=(c == 3),
                    )
                nc.scalar.activation(
                    dst[:, :], pq[:, :], AF.Identity, bias=b_sb[:, :], scale=scale
                )

            # q natural [l, dh] and v natural [m, dh] via PE transpose.
            for src, dst in ((q_hT, q_sb), (v_hT, v_sb)):
                for c in range(4):
                    pt = psB.tile([128, D], f32, tag="ptr", name="pt")
                    nc.tensor.transpose(
                        pt[:, :], src[:, c * 128 : (c + 1) * 128], ident[:D, :D]
                    )
                    nc.scalar.copy(dst[:, c, :], pt[:, :])

        # ---- Main loop over l-chunks ----
        rk_pool = ctx.enter_context(tc.tile_pool(name="rkp", bufs=3))
        rv_pool = ctx.enter_context(tc.tile_pool(name="rvp", bufs=3))
        small = ctx.enter_context(tc.tile_pool(name="small", bufs=2))
        outp = ctx.enter_context(tc.tile_pool(name="outp", bufs=2))

        for lc in range(4):
            lsl = slice(lc * 128, (lc + 1) * 128)

            # logits main term: q_h @ k_h^T -> [128 l, 512 m] in PSUM
            pl = psA.tile([128, L], f32, tag="logits")
            nc.tensor.matmul(
                pl[:, :], q_hT[:, lsl], k_hT[:, :], start=True, stop=True
            )

            # rel-k term on DVE
            lrel = small.tile([128, L], f32, tag="lrel")
            for g in range(NG):
                rkt = rk_pool.tile([128, G, D], f32, tag="rkt")
                nc.sync.dma_start(
                    rkt[:, :, :], rk[lsl, g * G : (g + 1) * G, :]
                )
                qb = q_sb[:, lc, :].unsqueeze(1).broadcast_to([128, G, D])
                nc.vector.tensor_tensor(
                    rkt[:, :, :], rkt[:, :, :], qb, OP.mult
                )
                nc.vector.tensor_reduce(
                    lrel[:, g * G : (g + 1) * G], rkt[:, :, :], AX.X, OP.add
                )

            logits = small.tile([128, L], f32, tag="logits_sb")
            nc.vector.tensor_tensor(logits[:, :], pl[:, :], lrel[:, :], OP.add)

            # softmax
            mx = small.tile([128, 1], f32, tag="mx")
            nmx = small.tile([128, 1], f32, tag="nmx")
            sm = small.tile([128, 1], f32, tag="sm")
            rs = small.tile([128, 1], f32, tag="rs")
            nc.vector.tensor_reduce(mx[:, :], logits[:, :], AX.X, OP.max)
            nc.vector.tensor_scalar_mul(nmx[:, :], mx[:, :], -1.0)
            nc.scalar.activation(
                attn_sb[:, lc, :], logits[:, :], AF.Exp, bias=nmx[:, :], scale=1.0
            )
            nc.vector.tensor_reduce(sm[:, :], attn_sb[:, lc, :], AX.X, OP.add)
            nc.vector.reciprocal(rs[:, :], sm[:, :])
            nc.vector.tensor_scalar_mul(attn_sb[:, lc, :], attn_sb[:, lc, :], rs[:, :])

            nc.sync.dma_start(attn_o[lsl, :], attn_sb[:, lc, :])

            # attn^T blocks for the context matmul
            for j in range(4):
                pt = psB.tile([128, 128], f32, tag="ptr")
                nc.tensor.transpose(
                    pt[:, :], attn_sb[:, lc, j * 128 : (j + 1) * 128], ident[:, :]
                )
                nc.scalar.copy(attnT[:, j, lsl], pt[:, :])

            # rel-v term on DVE
            ctx_rel = small.tile([128, D], f32, tag="ctx_rel")
            rtmp = small.tile([128, D], f32, tag="rtmp")
            for g in range(NG):
                rvt = rv_pool.tile([128, G, D], f32, tag="rvt")
                nc.sync.dma_start(
                    rvt[:, :, :], rv[lsl, g * G : (g + 1) * G, :]
                )
                ab = (
                    attn_sb[:, lc, g * G : (g + 1) * G]
                    .unsqueeze(2)
                    .broadcast_to([128, G, D])
                )
                nc.vector.tensor_tensor(rvt[:, :, :], rvt[:, :, :], ab, OP.mult)
                red_dst = ctx_rel if g == 0 else rtmp
                nc.vector.tensor_reduce(
                    red_dst[:, :],
                    rvt[:, :, :].rearrange("p m d -> p d m"),
                    AX.X,
                    OP.add,
                )
                if g > 0:
                    nc.vector.tensor_tensor(
                        ctx_rel[:, :], ctx_rel[:, :], rtmp[:, :], OP.add
                    )

            # context main term: attn @ v -> [128 l, 64]
            pc = psD.tile([128, D], f32, tag="pctx")
            for j in range(4):
                nc.tensor.matmul(
                    pc[:, :],
                    attnT[:, j, lsl],
                    v_sb[:, j, :],
                    start=(j == 0),
                    stop=(j == 3),
                )
            ctx_sb = small.tile([128, D], f32, tag="ctx_sb")
            nc.vector.tensor_tensor(ctx_sb[:, :], pc[:, :], ctx_rel[:, :], OP.add)

            # context^T [dh, l-slice]
            ptc = psB.tile([D, 128], f32, tag="ptr", name="ptc")
            nc.tensor.transpose(ptc[:, :], ctx_sb[:, :], ident[:, :])
            nc.scalar.copy(ctxT[:, lsl], ptc[:, :])

        # ---- Output projection partial: (ctx @ Wo)^T = Wo^T-slices @ ctxT ----
        for c in range(4):
            po = psA.tile([128, L], f32, tag="logits")
            nc.tensor.matmul(
                po[:, :],
                wo_sb[:, c * 128 : (c + 1) * 128],
                ctxT[:, :],
                start=True,
                stop=True,
            )
            osb = outp.tile([128, L], f32, tag="osb")
            nc.scalar.copy(osb[:, :], po[:, :])
            nc.sync.dma_start(pout_t[c * 128 : (c + 1) * 128, :], osb[:, :])

    nc.finalize()
    return nc


def _get_nc(repeat=1):
    key = ("nc", repeat)
    if key not in _cache:
        _cache[key] = _build(repeat)
    return _cache[key]


def _prep_in_maps(
    query, key, value, relative_k, relative_v, Wq, bq, Wk, bk, Wv, bv, Wo
):
    q = np.ascontiguousarray(np.asarray(query, np.float32).reshape(L, DM))
    k = np.ascontiguousarray(np.asarray(key, np.float32).reshape(L, DM))
    v = np.ascontiguousarray(np.asarray(value, np.float32).reshape(L, DM))
    rk = np.ascontiguousarray(np.asarray(relative_k, np.float32).reshape(H, L, L, D))
    rv = np.ascontiguousarray(np.asarray(relative_v, np.float32).reshape(H, L, L, D))
    Wq = np.asarray(Wq, np.float32)
    Wk = np.asarray(Wk, np.float32)
    Wv = np.asarray(Wv, np.float32)
    Wo = np.asarray(Wo, np.float32)
    bq = np.asarray(bq, np.float32)
    bk = np.asarray(bk, np.float32)
    bv = np.asarray(bv, np.float32)

    in_maps = []
    for h in range(H):
        sl = slice(h * D, (h + 1) * D)
        in_maps.append(
            {
                "xq": q,
                "xk": k,
                "xv": v,
                "wq": np.ascontiguousarray(Wq[:, sl]),
                "wk": np.ascontiguousarray(Wk[:, sl]),
                "wv": np.ascontiguousarray(Wv[:, sl]),
                "bqs": np.ascontiguousarray((bq[sl] / 8.0).reshape(D, 1)),
                "bks": np.ascontiguousarray(bk[sl].reshape(D, 1)),
                "bvs": np.ascontiguousarray(bv[sl].reshape(D, 1)),
                "wo": np.ascontiguousarray(Wo[sl, :]),
                "rk": rk[h],
                "rv": rv[h],
            }
        )
    return in_maps


def _assemble(res, bo):
    bo = np.asarray(bo, np.float32)
    attn = np.stack([r["attn_o"] for r in res.results])[None]
    out = sum(r["pout_t"].T for r in res.results) + bo
    return out[None].astype(np.float32), attn.astype(np.float32)


def kernel(
    query,
    key,
    value,
    relative_k,
    relative_v,
    Wq,
    bq,
    Wk,
    bk,
    Wv,
    bv,
    Wo,
    bo,
):
    from concourse.bass_utils import run_bass_kernel_spmd

    in_maps = _prep_in_maps(
        query, key, value, relative_k, relative_v, Wq, bq, Wk, bk, Wv, bv, Wo
    )
    res = run_bass_kernel_spmd(_get_nc(), in_maps, core_ids=list(range(H)))
    return _assemble(res, bo)


def profile(inputs, tmpdir=None):
    """Run once with NTFF tracing; returns (outputs, exec_time_ns, results)."""
    from concourse.bass_utils import run_bass_kernel_spmd

    in_maps = _prep_in_maps(
        **{k: v for k, v in inputs.items() if k not in ("bo",)}
    )
    res = run_bass_kernel_spmd(
        _get_nc(), in_maps, core_ids=list(range(H)), trace=True, tmpdir=tmpdir
    )
    outs = _assemble(res, inputs["bo"])
    return outs, res.exec_time_ns, res


# revision 13
# speedup vs baseline: 190.9146x; 1.4413x over previous
"""Multi-head attention with relative position embeddings, on 8 NeuronCores.

Sharding: tensor-parallel over heads. Core h handles head h: it gets the
full query/key/value plus the per-head slices of Wq/Wk/Wv/Wo and
relative_k/relative_v. Each core computes attn_h [512,512] and the partial
output projection context_h @ Wo_h [512,512] (transposed); the host sums the
8 partials (+ bo) and stacks the attn maps.

Per-core device schedule:
  - rel-k term: the host pre-arranges rel_k[h] as [256 l-pairs, 128, 512]
    where the 128 partitions hold (pair-member j, d). DVE multiplies each
    pair tile by q (per-partition tensor_scalar, 2x mode), then PE contracts
    the d axis with a constant [128,2] block-indicator matmul, accumulating
    into the same PSUM bank as the q@k^T logits.
  - rel-v term: streamed in natural [l, m, d] tiles; GpSimd does the
    broadcast multiply, DVE reduces over m.
  - PE also transposes q/k/v, does head projections, attn@v and the Wo
    partial. ACT does softmax exp and PSUM->SBUF copies.
"""

import sys

sys.path.insert(0, "/opt/trn_rl_repo")

import numpy as np

L = 512  # sequence length
DM = 512  # model dim
H = 8  # heads
D = 64  # dim per head
G = 64  # m-group size for rel-v streaming
NG = L // G  # 8 groups per chunk
NP = 8  # pairs per rel-k tile

# When True the relative_k / relative_v streams are shipped and processed in
# bfloat16 (halves the HBM traffic that dominates this kernel). The qk / attn
# / projection math stays fp32.
REL_BF16 = False

_cache = {}


def _build(repeat=1, rel_bf16=None):
    if rel_bf16 is None:
        rel_bf16 = REL_BF16
    from contextlib import ExitStack

    import concourse.bacc as bacc
    import concourse.tile as tile
    from concourse import mybir
    from concourse.masks import make_identity

    f32 = mybir.dt.float32
    bf16 = mybir.dt.bfloat16
    rel_dt = bf16 if rel_bf16 else f32
    AX = mybir.AxisListType
    OP = mybir.AluOpType
    AF = mybir.ActivationFunctionType

    nc = bacc.Bacc("TRN2", target_bir_lowering=False)

    xq = nc.dram_tensor("xq", [L, DM], f32, kind="ExternalInput")
    xk = nc.dram_tensor("xk", [L, DM], f32, kind="ExternalInput")
    xv = nc.dram_tensor("xv", [L, DM], f32, kind="ExternalInput")
    wq = nc.dram_tensor("wq", [DM, D], f32, kind="ExternalInput")
    wk = nc.dram_tensor("wk", [DM, D], f32, kind="ExternalInput")
    wv = nc.dram_tensor("wv", [DM, D], f32, kind="ExternalInput")
    bqs2 = nc.dram_tensor("bqs2", [128, 1], f32, kind="ExternalInput")
    bks = nc.dram_tensor("bks", [D, 1], f32, kind="ExternalInput")
    bvs = nc.dram_tensor("bvs", [D, 1], f32, kind="ExternalInput")
    wo = nc.dram_tensor("wo", [D, DM], f32, kind="ExternalInput")
    rkp = nc.dram_tensor("rkp", [L // 2, 128, L], rel_dt, kind="ExternalInput")
    rv = nc.dram_tensor("rv", [L, L, D], rel_dt, kind="ExternalInput")

    attn_o = nc.dram_tensor("attn_o", [L, L], f32, kind="ExternalOutput")
    pout_t = nc.dram_tensor("pout_t", [DM, L], f32, kind="ExternalOutput")

    with ExitStack() as ctx:
        tc = ctx.enter_context(tile.TileContext(nc))
        const = ctx.enter_context(tc.tile_pool(name="const", bufs=1))
        persist = ctx.enter_context(tc.tile_pool(name="persist", bufs=1))
        psA = ctx.enter_context(tc.tile_pool(name="psA", bufs=2, space="PSUM"))
        psB = ctx.enter_context(tc.tile_pool(name="psB", bufs=2, space="PSUM"))
        psD = ctx.enter_context(tc.tile_pool(name="psD", bufs=2, space="PSUM"))

        ident = const.tile([128, 128], f32)
        make_identity(nc, ident[:, :])

        # Banded indicator for the rel-k pair reduction. Slicing
        # band[:, 126-2*pi : 254-2*pi] gives a [128,128] stationary whose
        # only nonzero columns map partition-half j to output row 2*pi+j;
        # all other output rows accumulate exact zeros.
        band = const.tile([128, 254], rel_dt)
        nc.vector.memset(band[:, :], 0.0)
        nc.vector.memset(band[0:64, 126:127], 1.0)
        nc.vector.memset(band[64:128, 127:128], 1.0)

        wq_sb = const.tile([128, 4, D], f32)
        wk_sb = const.tile([128, 4, D], f32)
        wv_sb = const.tile([128, 4, D], f32)
        nc.sync.dma_start(wq_sb[:, :, :], wq[:, :].rearrange("(c p) d -> p c d", c=4))
        nc.sync.dma_start(wk_sb[:, :, :], wk[:, :].rearrange("(c p) d -> p c d", c=4))
        nc.sync.dma_start(wv_sb[:, :, :], wv[:, :].rearrange("(c p) d -> p c d", c=4))
        wo_sb = const.tile([D, DM], f32)
        nc.sync.dma_start(wo_sb[:, :], wo[:, :])
        bq2_sb = const.tile([128, 1], f32)
        bk_sb = const.tile([D, 1], f32)
        bv_sb = const.tile([D, 1], f32)
        nc.sync.dma_start(bq2_sb[:, :], bqs2[:, :])
        nc.sync.dma_start(bk_sb[:, :], bks[:, :])
        nc.sync.dma_start(bv_sb[:, :], bvs[:, :])

        # Persistent head tensors.
        q_hT = persist.tile([D, L], f32)  # q^T/sqrt(D): [dh, l]
        k_hT = persist.tile([D, L], f32)  # k^T: [dh, m]
        # q in stacked pair layout: partition (j,d) = q[2p+j, d]/sqrt(D)
        q_st = persist.tile([128, 4, D], f32)  # [.., lc, pair-in-chunk]
        v_sb = persist.tile([128, 4, D], f32)  # v natural: [m-part, mc, dh]
        attn_sb = persist.tile([128, 4, L], f32)  # attn: [l-part, lc, m]
        attnT = persist.tile([128, 4, L], f32)  # attn^T: [m-part, mc, l]
        ctxT = persist.tile([D, L], f32)  # context^T: [dh, l]

        # Optional on-device repeat loop for benchmarking (repeat>1).
        rep_cm = tc.For_i(0, repeat, 1) if repeat > 1 else None
        if rep_cm is not None:
            ctx.enter_context(rep_cm)

        # ---- Stage 0: load q/k/v, transpose, project ----
        with tc.tile_pool(name="s0", bufs=1) as s0:
            x_sb = {}
            xT = {}
            for name, t in (("q", xq), ("k", xk), ("v", xv)):
                xs = s0.tile([128, 4, DM], f32, tag=f"x_{name}", name=f"x_{name}")
                nc.sync.dma_start(
                    xs[:, :, :], t[:, :].rearrange("(c p) d -> p c d", c=4)
                )
                x_sb[name] = xs
                xT[name] = s0.tile(
                    [128, 4, L], f32, tag=f"xT_{name}", name=f"xT_{name}"
                )
            for name in ("q", "k", "v"):
                for c in range(4):  # l-chunk
                    for j in range(4):  # dm-chunk
                        pt = psB.tile([128, 128], f32, tag="ptr")
                        nc.tensor.transpose(
                            pt[:, :],
                            x_sb[name][:, c, j * 128 : (j + 1) * 128],
                            ident[:, :],
                        )
                        nc.scalar.copy(
                            xT[name][:, j, c * 128 : (c + 1) * 128], pt[:, :]
                        )

            # Projections into transposed head layout [dh, l].
            v_hT = s0.tile([D, L], f32)
            for dst, w_sb, xn, b_sb, scale in (
                (q_hT, wq_sb, "q", None, 0.125),
                (k_hT, wk_sb, "k", bk_sb, 1.0),
                (v_hT, wv_sb, "v", bv_sb, 1.0),
            ):
                pq = psA.tile([D, L], f32, tag="big", name="pq")
                for c in range(4):
                    nc.tensor.matmul(
                        pq[:, :],
                        w_sb[:, c, :],
                        xT[xn][:, c, :],
                        start=(c == 0),
                        stop=(c == 3),
                    )
                if b_sb is None:
                    nc.scalar.activation(
                        dst[:, :], pq[:, :], AF.Copy, bias=0.0, scale=scale
                    )
                else:
                    nc.scalar.activation(
                        dst[:, :], pq[:, :], AF.Identity, bias=b_sb[:, :], scale=scale
                    )

            # q in stacked pair layout, one [128, 64] block per l-chunk:
            # out[(j,d), p] = sum_dm wq[dm, d] * queryT[dm, lc*128 + 2p + j]
            for lc in range(4):
                pqs = psD.tile([128, D], f32, tag="pctx", name="pqs")
                for j in range(2):
                    for c in range(4):
                        rhs = xT["q"][:, c, :].rearrange(
                            "p (a two) -> p two a", two=2
                        )[:, j, lc * 64 : (lc + 1) * 64]
                        nc.tensor.matmul(
                            pqs[j * 64 : (j + 1) * 64, :],
                            wq_sb[:, c, :],
                            rhs,
                            start=(c == 0),
                            stop=(c == 3),
                        )
                nc.scalar.activation(
                    q_st[:, lc, :],
                    pqs[:, :],
                    AF.Identity,
                    bias=bq2_sb[:, :],
                    scale=0.125,
                )

            # v natural [m, dh] via PE transpose.
            for c in range(4):
                pt = psB.tile([128, D], f32, tag="ptr", name="pt")
                nc.tensor.transpose(
                    pt[:, :], v_hT[:, c * 128 : (c + 1) * 128], ident[:D, :D]
                )
                nc.scalar.copy(v_sb[:, c, :], pt[:, :])

        # ---- Main loop over l-chunks ----
        rk_pool = ctx.enter_context(tc.tile_pool(name="rk_pool", bufs=3))
        rv_pool = ctx.enter_context(tc.tile_pool(name="rv_pool", bufs=3))
        small = ctx.enter_context(tc.tile_pool(name="small", bufs=2))
        outp = ctx.enter_context(tc.tile_pool(name="outp", bufs=2))

        for lc in range(4):
            lsl = slice(lc * 128, (lc + 1) * 128)

            # logits: q@k^T then rel-k pair reductions, same PSUM bank.
            pl = psA.tile([128, L], f32, tag="big", name="pl")
            nc.tensor.matmul(
                pl[:, :], q_hT[:, lsl], k_hT[:, :], start=True, stop=True
            )

            for t in range(64 // NP):  # 8 tiles of NP=8 pairs
                rkt = rk_pool.tile([128, NP, L], rel_dt, tag="rkt")
                nc.sync.dma_start(
                    rkt[:, :, :],
                    rkp[lc * 64 + t * NP : lc * 64 + (t + 1) * NP, :, :].rearrange(
                        "a p m -> p a m"
                    ),
                )
                for i in range(NP):
                    pi = t * NP + i  # pair within chunk
                    nc.vector.tensor_scalar_mul(
                        rkt[:, i, :], rkt[:, i, :], q_st[:, lc, pi : pi + 1]
                    )
                    nc.tensor.matmul(
                        pl[:, :],
                        band[:, 126 - 2 * pi : 254 - 2 * pi],
                        rkt[:, i, :],
                        start=False,
                        stop=True,
                        skip_group_check=True,
                    )

            # softmax straight out of PSUM
            mx = small.tile([128, 1], f32, tag="mx")
            nmx = small.tile([128, 1], f32, tag="nmx")
            sm = small.tile([128, 1], f32, tag="sm")
            rs = small.tile([128, 1], f32, tag="rs")
            nc.vector.tensor_reduce(mx[:, :], pl[:, :], AX.X, OP.max)
            nc.vector.tensor_scalar_mul(nmx[:, :], mx[:, :], -1.0)
            nc.scalar.activation(
                attn_sb[:, lc, :], pl[:, :], AF.Exp, bias=nmx[:, :], scale=1.0
            )
            nc.vector.tensor_reduce(sm[:, :], attn_sb[:, lc, :], AX.X, OP.add)
            nc.vector.reciprocal(rs[:, :], sm[:, :])
            nc.vector.tensor_scalar_mul(attn_sb[:, lc, :], attn_sb[:, lc, :], rs[:, :])

            nc.sync.dma_start(attn_o[lsl, :], attn_sb[:, lc, :])

            # attn^T blocks for the context matmul
            for j in range(4):
                pt = psB.tile([128, 128], f32, tag="ptr")
                nc.tensor.transpose(
                    pt[:, :], attn_sb[:, lc, j * 128 : (j + 1) * 128], ident[:, :]
                )
                nc.scalar.copy(attnT[:, j, lsl], pt[:, :])

            # rel-v term: GpSimd multiply, DVE reduce over m.
            if rel_bf16:
                attn_rel = small.tile([128, L], bf16, tag="attn_bf")
                nc.vector.tensor_copy(attn_rel[:, :], attn_sb[:, lc, :])
            else:
                attn_rel = attn_sb[:, lc, :]
            ctx_rel = small.tile([128, D], f32, tag="ctx_rel")
            rtmp = small.tile([128, D], f32, tag="rtmp")
            for g in range(NG):
                rvt = rv_pool.tile([128, G, D], rel_dt, tag="rvt")
                nc.sync.dma_start(
                    rvt[:, :, :], rv[lsl, g * G : (g + 1) * G, :]
                )
                ab = (
                    attn_rel[:, g * G : (g + 1) * G]
                    .unsqueeze(2)
                    .broadcast_to([128, G, D])
                )
                nc.gpsimd.tensor_tensor(rvt[:, :, :], rvt[:, :, :], ab, OP.mult)
                red_dst = ctx_rel if g == 0 else rtmp
                nc.vector.tensor_reduce(
                    red_dst[:, :],
                    rvt[:, :, :].rearrange("p m d -> p d m"),
                    AX.X,
                    OP.add,
                )
                if g > 0:
                    nc.vector.tensor_tensor(
                        ctx_rel[:, :], ctx_rel[:, :], rtmp[:, :], OP.add
                    )

            # context main term: attn @ v -> [128 l, 64]
            pc = psD.tile([128, D], f32, tag="pctx")
            for j in range(4):
                nc.tensor.matmul(
                    pc[:, :],
                    attnT[:, j, lsl],
                    v_sb[:, j, :],
                    start=(j == 0),
                    stop=(j == 3),
                )
            ctx_sb = small.tile([128, D], f32, tag="ctx_sb")
            nc.vector.tensor_tensor(ctx_sb[:, :], pc[:, :], ctx_rel[:, :], OP.add)

            # context^T [dh, l-slice]
            ptc = psB.tile([D, 128], f32, tag="ptr", name="ptc")
            nc.tensor.transpose(ptc[:, :], ctx_sb[:, :], ident[:, :])
            nc.scalar.copy(ctxT[:, lsl], ptc[:, :])

        # ---- Output projection partial: (ctx @ Wo)^T = Wo^T-slices @ ctxT ----
        for c in range(4):
            po = psA.tile([128, L], f32, tag="big", name="po")
            nc.tensor.matmul(
                po[:, :],
                wo_sb[:, c * 128 : (c + 1) * 128],
                ctxT[:, :],
                start=True,
                stop=True,
            )
            osb = outp.tile([128, L], f32, tag="osb")
            nc.scalar.copy(osb[:, :], po[:, :])
            nc.sync.dma_start(pout_t[c * 128 : (c + 1) * 128, :], osb[:, :])

    nc.finalize()
    return nc


def _get_nc(repeat=1, rel_bf16=None):
    if rel_bf16 is None:
        rel_bf16 = REL_BF16
    key = ("nc", repeat, rel_bf16)
    if key not in _cache:
        _cache[key] = _build(repeat, rel_bf16)
    return _cache[key]


def _prep_in_maps(
    query, key, value, relative_k, relative_v, Wq, bq, Wk, bk, Wv, bv, Wo,
    rel_bf16=None,
):
    import ml_dtypes

    if rel_bf16 is None:
        rel_bf16 = REL_BF16
    rel_np = ml_dtypes.bfloat16 if rel_bf16 else np.float32
    q = np.ascontiguousarray(np.asarray(query, np.float32).reshape(L, DM))
    k = np.ascontiguousarray(np.asarray(key, np.float32).reshape(L, DM))
    v = np.ascontiguousarray(np.asarray(value, np.float32).reshape(L, DM))
    rk = np.asarray(relative_k, np.float32).reshape(H, L, L, D)
    rv = np.ascontiguousarray(np.asarray(relative_v, np.float32).reshape(H, L, L, D))
    Wq = np.asarray(Wq, np.float32)
    Wk = np.asarray(Wk, np.float32)
    Wv = np.asarray(Wv, np.float32)
    Wo = np.asarray(Wo, np.float32)
    bq = np.asarray(bq, np.float32)
    bk = np.asarray(bk, np.float32)
    bv = np.asarray(bv, np.float32)

    in_maps = []
    for h in range(H):
        sl = slice(h * D, (h + 1) * D)
        # rel_k pair layout: [pair, (j,d), m] with partition index j*64+d,
        # value rk[2p+j, m, d].
        rkp = np.ascontiguousarray(
            rk[h].reshape(L // 2, 2, L, D).transpose(0, 1, 3, 2).astype(rel_np)
        ).reshape(L // 2, 128, L)
        bq_h = bq[sl]
        in_maps.append(
            {
                "xq": q,
                "xk": k,
                "xv": v,
                "wq": np.ascontiguousarray(Wq[:, sl]),
                "wk": np.ascontiguousarray(Wk[:, sl]),
                "wv": np.ascontiguousarray(Wv[:, sl]),
                "bqs2": np.ascontiguousarray(
                    np.concatenate([bq_h, bq_h]).reshape(128, 1) / 8.0
                ),
                "bks": np.ascontiguousarray(bk[sl].reshape(D, 1)),
                "bvs": np.ascontiguousarray(bv[sl].reshape(D, 1)),
                "wo": np.ascontiguousarray(Wo[sl, :]),
                "rkp": rkp,
                "rv": np.ascontiguousarray(rv[h].astype(rel_np)),
            }
        )
    return in_maps


def _assemble(res, bo):
    bo = np.asarray(bo, np.float32)
    attn = np.stack([r["attn_o"] for r in res.results])[None]
    out = sum(r["pout_t"].T for r in res.results) + bo
    return out[None].astype(np.float32), attn.astype(np.float32)


def kernel(
    query,
    key,
    value,
    relative_k,
    relative_v,
    Wq,
    bq,
    Wk,
    bk,
    Wv,
    bv,
    Wo,
    bo,
):
    from concourse.bass_utils import run_bass_kernel_spmd

    in_maps = _prep_in_maps(
        query, key, value, relative_k, relative_v, Wq, bq, Wk, bk, Wv, bv, Wo
    )
    res = run_bass_kernel_spmd(_get_nc(), in_maps, core_ids=list(range(H)))
    return _assemble(res, bo)
